# revision 1
# baseline (speedup 1.0000x reference)
"""Trainium2 Bass kernel: 16-head MHA (S=4096, D=1024) sharded 2 heads/core over 8 cores.

Per-core plan (heads h0=2c, h1=2c+1):
  - inputs: QT/KT/VT [D, S] bf16 (host-transposed, shared by all cores) + per-core
    weight slices pre-packed to SBUF layouts.
  - phase 1: projections.  qT/kT computed as [dk, S] on partitions 0:64 (per head),
    scaled by 1/sqrt(dk) and biased.  v computed directly in [t, j] layout ([128, 128]
    per t-block, j = 2 heads x 64) with a ones column appended per head for the
    softmax denominator.  v bias is exactly equivalent to a constant output shift
    (softmax rows sum to 1), so it is applied on the host.
  - phase 2 (fused per s-block): scoresT tiles [t=128, s<=512] = kT-block.T @ qT-block
    in PSUM; exp on ScalarE (no max subtraction needed: scores ~ N(0,1), |s| < 10);
    P tiles fp16 in SBUF (HW-measured: ScalarE writing bf16 to SBUF is 2x slower
    than fp16/fp32, and f32r matmul moving operands stream 3.3x slower than
    16-bit, so fp16 P gets both fast exp and full-rate PV); oT[65, 512] +=
    [v|1].T @ P accumulated over t with PV issued 3 groups behind QK; row 64 is
    the softmax denominator l.  r = 1/l via fast-reciprocal DVE op (SBUF-staged:
    the custom-DVE op mis-reads PSUM); broadcast r across 64 partitions with a
    K=1 fp32 matmul; xT = oT * r (bf16).  Output projection out[s,m] =
    sum_h xT_h.T @ WoT_h accumulated K=64 twice, copied to SBUF, DMA'd out.
  - host: sum the 8 partial outputs + bias terms.
"""

import sys

for _p in ("/opt/trn_rl_repo",):
    if _p not in sys.path:
        sys.path.insert(0, _p)

import numpy as np
import ml_dtypes

import concourse.bass as bass  # noqa: F401
import concourse.mybir as mybir
import concourse.tile as tile
from concourse import bacc
from concourse.bass_utils import run_bass_kernel_spmd

P = 128
S = 4096
D = 1024
H = 16
DK = 64
HL = 2            # heads per core
NC = 8            # cores
SB = 512          # s-block width
NSB = S // SB     # 8
TB = 128          # t-block (scores partition dim)
NTB = S // TB     # 32
DO = D // P       # 8 d-chunks
# t-block group sizes per (head, s-block); alternate 2/4 so the two PSUM score
# slots (2-bank and 4-bank) ping-pong and ScalarE exp never waits on TensorE.
GROUPS = [2, 4, 2, 4, 2, 4, 2, 4, 2, 4, 2]
assert sum(GROUPS) == NTB

F32 = mybir.dt.float32
F32R = mybir.dt.float32r
F16 = mybir.dt.float16
BF16 = mybir.dt.bfloat16
BF16_NP = ml_dtypes.bfloat16


def build_nc(reps: int = 1, loop_n: int = 0, phase1: bool = True, phase2: bool = True):
    """Build the per-core Bass kernel.

    `reps`: unrolled repetitions of the body.  `loop_n` > 0 instead wraps the
    body in a hardware For loop with that many iterations (timing builds).
    """
    from contextlib import ExitStack

    nc = bacc.Bacc("TRN2", target_bir_lowering=False, debug=False, num_devices=NC)
    qt = nc.dram_tensor("qt", [D, S], BF16, kind="ExternalInput").ap()
    kt = nc.dram_tensor("kt", [D, S], BF16, kind="ExternalInput").ap()
    vt = nc.dram_tensor("vt", [D, S], BF16, kind="ExternalInput").ap()
    wqk = nc.dram_tensor("wqk", [P, DO, 2, HL, DK], BF16, kind="ExternalInput").ap()
    wv = nc.dram_tensor("wv", [P, DO, P], BF16, kind="ExternalInput").ap()
    wot = nc.dram_tensor("wot", [DK, HL, D], BF16, kind="ExternalInput").ap()
    bqk = nc.dram_tensor("bqk", [DK, 4], F32, kind="ExternalInput").ap()
    out = nc.dram_tensor("out", [S, D], F32, kind="ExternalOutput").ap()
    # The neuron NEFF cache keys on the HLO signature only (the Bass IR rides
    # out-of-band), so distinct kernel builds with identical I/O signatures can
    # collide with stale cache entries.  A version/config-sized dummy output
    # makes every build's signature unique.
    _KVER = 16
    vw = 16 + 64 * _KVER + 4 * loop_n + reps + (0 if phase1 else 1) + (0 if phase2 else 2)
    ver = nc.dram_tensor("ver", [1, vw], F32, kind="ExternalOutput").ap()

    qt_r = qt.rearrange("(o p) s -> p o s", p=P)
    kt_r = kt.rearrange("(o p) s -> p o s", p=P)
    vt_r = vt.rearrange("(o p) s -> p o s", p=P)
    out_r = out.rearrange("(so p) m -> p so m", p=P)  # [128, 32, 1024]

    with tile.TileContext(nc) as tc, ExitStack() as ctx:
        const = ctx.enter_context(tc.tile_pool(name="const", bufs=1))
        pers = ctx.enter_context(tc.tile_pool(name="pers", bufs=1))
        pin = ctx.enter_context(tc.tile_pool(name="pin", bufs=2))
        pP = ctx.enter_context(tc.tile_pool(name="pP", bufs=5))
        pout = ctx.enter_context(tc.tile_pool(name="pout", bufs=3))
        prr = ctx.enter_context(tc.tile_pool(name="prr", bufs=2))
        # PSUM: sA 4 banks + sB 2 banks + oT 1 bank + pv 1 bank = 8 banks
        psA = ctx.enter_context(tc.tile_pool(name="psA", bufs=1, space="PSUM"))
        psB = ctx.enter_context(tc.tile_pool(name="psB", bufs=1, space="PSUM"))
        psO = ctx.enter_context(tc.tile_pool(name="psO", bufs=1, space="PSUM"))
        psV = ctx.enter_context(tc.tile_pool(name="psV", bufs=1, space="PSUM"))

        wqk_sb = const.tile([P, DO, 2, HL, DK], BF16)
        nc.sync.dma_start(wqk_sb[:], wqk)
        wv_sb = const.tile([P, DO, P], BF16)
        nc.sync.dma_start(wv_sb[:], wv)
        wot_sb = const.tile([DK, HL, D], BF16)
        nc.sync.dma_start(wot_sb[:], wot)
        bqk_sb = const.tile([DK, 4], F32)
        nc.sync.dma_start(bqk_sb[:], bqk)
        ones_sb = const.tile([1, DK], F32)
        nc.vector.memset(ones_sb[:], 1.0)
        ver_sb = const.tile([1, vw], F32)
        nc.vector.memset(ver_sb[:], float(vw))
        nc.sync.dma_start(ver, ver_sb[:])

        def body():
            qT = pers.tile([DK, HL, S], BF16, tag="qT", name="qT")
            kT = pers.tile([DK, HL, S], BF16, tag="kT", name="kT")
            vx0 = pers.tile([P, NTB, DK + 1], F16, tag="vx0", name="vx0")
            vx1 = pers.tile([P, NTB, DK + 1], F16, tag="vx1", name="vx1")
            xT = pers.tile([DK, HL, S], BF16, tag="xT", name="xT")
            nc.vector.memset(vx0[:, :, DK], 1.0)
            nc.vector.memset(vx1[:, :, DK], 1.0)

            # ---------------- phase 1: projections ----------------
            for sb in range(NSB if phase1 else 0):
                s0 = sb * SB
                qs = pin.tile([P, DO, SB], BF16, tag="qs", name="qs")
                nc.sync.dma_start(qs[:], qt_r[:, :, s0 : s0 + SB])
                ks = pin.tile([P, DO, SB], BF16, tag="ks", name="ks")
                nc.sync.dma_start(ks[:], kt_r[:, :, s0 : s0 + SB])
                vs = pin.tile([P, DO, SB], BF16, tag="vs", name="vs")
                nc.sync.dma_start(vs[:], vt_r[:, :, s0 : s0 + SB])

                # q/k projections: psum slices [64, 512] all on partitions 0:64
                pq = psA.tile([P, 4 * SB], F32, tag="sA", name="pq")
                pk = psB.tile([P, 2 * SB], F32, tag="sB", name="pk")
                for h in range(HL):
                    pqh = pq[0:DK, h * SB : (h + 1) * SB]
                    pkh = pk[0:DK, h * SB : (h + 1) * SB]
                    for o in range(DO):
                        nc.tensor.matmul(
                            pqh, wqk_sb[:, o, 0, h, :], qs[:, o],
                            start=(o == 0), stop=(o == DO - 1),
                        )
                    for o in range(DO):
                        nc.tensor.matmul(
                            pkh, wqk_sb[:, o, 1, h, :], ks[:, o],
                            start=(o == 0), stop=(o == DO - 1),
                        )
                    nc.vector.tensor_scalar(
                        qT[:, h, s0 : s0 + SB], pqh,
                        bqk_sb[:, h : h + 1], 0.125,
                        mybir.AluOpType.add, mybir.AluOpType.mult,
                    )
                    nc.vector.tensor_scalar(
                        kT[:, h, s0 : s0 + SB], pkh,
                        bqk_sb[:, 2 + h : 3 + h], None,
                        mybir.AluOpType.add,
                    )

                # v directly in [t, j] layout, j = h0 dk | h1 dk
                for tb in range(SB // TB):
                    # alternate between the two 1-bank slots (oT is idle in phase 1)
                    if tb % 2 == 0:
                        pv = psV.tile([P, SB], F32, tag="pv", name="pvv")
                    else:
                        pv = psO.tile([P, SB], F32, tag="oT", name="pvo")
                    pvv = pv[:, 0:P]
                    for o in range(DO):
                        nc.tensor.matmul(
                            pvv, vs[:, o, tb * TB : (tb + 1) * TB], wv_sb[:, o],
                            start=(o == 0), stop=(o == DO - 1),
                        )
                    tbg = sb * (SB // TB) + tb
                    nc.vector.tensor_copy(vx0[:, tbg, 0:DK], pvv[:, 0:DK])
                    nc.vector.tensor_copy(vx1[:, tbg, 0:DK], pvv[:, DK:P])

            # -------- phase 2: attention + output projection, fused per s-block -----
            for sb in range(NSB if phase2 else 0):
                s0 = sb * SB
                for h in range(HL):
                    vx = vx0 if h == 0 else vx1
                    oT = psO.tile([DK + 1, SB], F32, tag="oT", name="oT")
                    # software-pipelined: PV for group g issues a few groups late
                    # so the PE always has independent QK work while ACT does exp
                    starts = []
                    t = 0
                    for g in GROUPS:
                        starts.append(t)
                        t += g
                    pts = [None] * len(GROUPS)
                    NG = len(GROUPS)
                    PV_LAG = 3

                    def do_pv(gi):
                        g, gs = GROUPS[gi], starts[gi]
                        for i in range(g):
                            tb = gs + i
                            nc.tensor.matmul(
                                oT, vx[:, tb, :], pts[gi][:, i * SB : (i + 1) * SB],
                                start=(tb == 0), stop=(tb == NTB - 1),
                            )

                    for gi, g in enumerate(GROUPS):
                        gs = starts[gi]
                        pool, width = (psB, 2 * SB) if g == 2 else (psA, 4 * SB)
                        sc = pool.tile([P, width], F32, tag=("sB" if g == 2 else "sA"), name="sc")
                        for i in range(g):
                            tb = gs + i
                            nc.tensor.matmul(
                                sc[:, i * SB : (i + 1) * SB],
                                kT[:, h, tb * TB : (tb + 1) * TB],
                                qT[:, h, s0 : s0 + SB],
                                start=True, stop=True,
                            )
                        pt = pP.tile([P, 4 * SB], F16, tag="P", name="pt")
                        nc.scalar.activation(
                            pt[:, : g * SB], sc[:, : g * SB],
                            mybir.ActivationFunctionType.Exp,
                        )
                        pts[gi] = pt
                        if gi >= PV_LAG:
                            do_pv(gi - PV_LAG)
                    for gi in range(NG - PV_LAG, NG):
                        do_pv(gi)

                    # r = 1/l ; broadcast over 64 partitions ; xT = oT * r
                    r_t = prr.tile([1, SB], F32, tag="r", name="r_t")
                    r_s = prr.tile([1, SB], F32, tag="rs", name="r_s")
                    l_t = prr.tile([1, SB], F32, tag="lt", name="l_t")
                    # custom-DVE reciprocal mis-reads PSUM operands: stage l in SBUF
                    nc.vector.tensor_copy(l_t[:], oT[DK : DK + 1, :])
                    nc.vector.reciprocal_approx_accurate(r_t[:], l_t[:], r_s[:])
                    bc = psV.tile([DK, SB], F32, tag="pv", name="bc")
                    nc.tensor.matmul(bc, ones_sb[:], r_t[:], start=True, stop=True)
                    # DVE has a single PSUM read port: stage the broadcast in SBUF
                    bc_sb = prr.tile([DK, SB], F32, tag="bcs", name="bc_sb")
                    nc.vector.tensor_copy(bc_sb[:], bc)
                    nc.vector.tensor_tensor(
                        xT[:, h, s0 : s0 + SB], oT[0:DK, :], bc_sb[:], mybir.AluOpType.mult
                    )

                # output projection for this s-block
                for si in range(SB // P):
                    so = sb * (SB // P) + si
                    for mb in range(2):
                        m0 = mb * SB
                        op = psV.tile([P, SB], F32, tag="pv", name="op")
                        for h in range(HL):
                            nc.tensor.matmul(
                                op, xT[:, h, so * P : (so + 1) * P],
                                wot_sb[:, h, m0 : m0 + SB],
                                start=(h == 0), stop=(h == HL - 1),
                            )
                        ob = pout.tile([P, SB], F32, tag="ob", name="ob")
                        nc.vector.tensor_copy(ob[:], op)
                        nc.sync.dma_start(out_r[:, so, m0 : m0 + SB], ob[:])

        if loop_n > 0:
            with tc.For_i(0, loop_n, 1):
                body()
        else:
            for _ in range(reps):
                body()

    nc.finalize()
    return nc


def _pack_core_inputs(c, QT, KT, VT, Wq, bq, Wk, bk, Wv, Wo):
    """Per-core input dict (core c owns heads 2c, 2c+1)."""
    h0 = HL * c
    wq = Wq[h0 : h0 + HL].reshape(HL, DO, P, DK).transpose(2, 1, 0, 3)  # [p,o,h,dk]
    wk = Wk[h0 : h0 + HL].reshape(HL, DO, P, DK).transpose(2, 1, 0, 3)
    wqk = np.stack([wq, wk], axis=2).astype(BF16_NP)  # [p, o, qk, h, dk]
    wv = (
        Wv[h0 : h0 + HL].reshape(HL, DO, P, DK).transpose(2, 1, 0, 3).reshape(P, DO, P)
    ).astype(BF16_NP)
    wot = (
        Wo[:, h0 * DK : (h0 + HL) * DK].reshape(D, HL, DK).transpose(2, 1, 0)
    ).astype(BF16_NP)  # [dk, h, m]
    bqk = np.stack(
        [bq[h0], bq[h0 + 1], bk[h0], bk[h0 + 1]], axis=1
    ).astype(np.float32)  # [64, 4]
    return {
        "qt": QT, "kt": KT, "vt": VT,
        "wqk": np.ascontiguousarray(wqk),
        "wv": np.ascontiguousarray(wv),
        "wot": np.ascontiguousarray(wot),
        "bqk": np.ascontiguousarray(bqk),
    }


def make_in_maps(Q, K, V, Wq, bq, Wk, bk, Wv, bv, Wo, bo):
    QT = np.ascontiguousarray(Q.T).astype(BF16_NP)
    KT = np.ascontiguousarray(K.T).astype(BF16_NP)
    VT = np.ascontiguousarray(V.T).astype(BF16_NP)
    return [
        _pack_core_inputs(c, QT, KT, VT, Wq, bq, Wk, bk, Wv, Wo) for c in range(NC)
    ]


def host_combine(partials, Wq, bv, Wo, bo):
    total = np.zeros((S, D), np.float32)
    for p in partials:
        total += p
    # v-bias passes through softmax exactly as +bv on the concat features
    total += bv.reshape(-1).astype(np.float32) @ Wo.T.astype(np.float32) + bo
    return total


_NC_CACHE = {}


def _get_nc(reps=1):
    if reps not in _NC_CACHE:
        _NC_CACHE[reps] = build_nc(reps)
    return _NC_CACHE[reps]


def kernel(Q, K, V, Wq, bq, Wk, bk, Wv, bv, Wo, bo):
    args = [np.asarray(x) for x in (Q, K, V, Wq, bq, Wk, bk, Wv, bv, Wo, bo)]
    Q, K, V, Wq, bq, Wk, bk, Wv, bv, Wo, bo = args
    nc = _get_nc()
    in_maps = make_in_maps(Q, K, V, Wq, bq, Wk, bk, Wv, bv, Wo, bo)
    res = run_bass_kernel_spmd(nc, in_maps, core_ids=list(range(NC)))
    partials = [res.results[c]["out"] for c in range(NC)]
    return host_combine(partials, Wq, bv, Wo, bo)



# revision 11
# speedup vs baseline: 1.1035x; 1.1035x over previous
"""Trainium2 Bass kernel: 16-head MHA (S=4096, D=1024) sharded 2 heads/core over 8 cores.

Per-core plan (heads h0=2c, h1=2c+1):
  - inputs: QT/KT/VT [D, S] bf16 (host-transposed, shared by all cores) + per-core
    weight slices pre-packed to SBUF layouts.
  - phase 1: projections.  qT/kT computed as [j, S] with j = h*64+dk on all 128
    partitions (both heads in ONE matmul stream per d-chunk: M=128 stationary,
    halving projection matmul time vs per-head M=64), scaled by 1/sqrt(dk) and
    biased with a single [128,1] tensor_scalar.  v computed directly in [t, j]
    layout ([128, 128] per t-block, j = 2 heads x 64) with a ones column appended
    per head for the softmax denominator.  v bias is exactly equivalent to a
    constant output shift (softmax rows sum to 1), so it is applied on the host.
    Per-head attention slices qT/kT at base partition h*64 (PE row groups {0,64}).
  - phase 2 (fused per s-block): scoresT tiles [t=128, s<=512] = kT-block.T @ qT-block
    in PSUM; exp on ScalarE (no max subtraction needed: scores ~ N(0,1), |s| < 10);
    P tiles fp16 in SBUF (HW-measured: ScalarE writing bf16 to SBUF is 2x slower
    than fp16/fp32, and f32r matmul moving operands stream 3.3x slower than
    16-bit, so fp16 P gets both fast exp and full-rate PV); oT[65, 512] +=
    [v|1].T @ P accumulated over t with PV issued 3 groups behind QK; row 64 is
    the softmax denominator l.  r = 1/l via fast-reciprocal DVE op (SBUF-staged:
    the custom-DVE op mis-reads PSUM); broadcast r across 64 partitions with a
    K=1 fp32 matmul; xT = oT * r (bf16) written into xT2 [j, S] at partition
    h*64.  Output projection out[s,m] = xT2_block.T @ WoT2 as a single K=128
    matmul per (128-row, 512-col) tile, copied to SBUF, DMA'd out.
  - host: sum the 8 partial outputs + bias terms.
"""

import sys

for _p in ("/opt/trn_rl_repo",):
    if _p not in sys.path:
        sys.path.insert(0, _p)

import numpy as np
import ml_dtypes

import concourse.bass as bass  # noqa: F401
import concourse.mybir as mybir
import concourse.tile as tile
from concourse import bacc
from concourse.bass_utils import run_bass_kernel_spmd

P = 128
S = 4096
D = 1024
H = 16
DK = 64
HL = 2            # heads per core
NC = 8            # cores
SB = 512          # s-block width
NSB = S // SB     # 8
TB = 128          # t-block (scores partition dim)
NTB = S // TB     # 32
DO = D // P       # 8 d-chunks
# t-block group sizes per (head, s-block); alternate 2/4 so the two PSUM score
# slots (2-bank and 4-bank) ping-pong and ScalarE exp never waits on TensorE.
GROUPS = [2, 4, 2, 4, 2, 4, 2, 4, 2, 4, 2]
assert sum(GROUPS) == NTB

F32 = mybir.dt.float32
F32R = mybir.dt.float32r
F16 = mybir.dt.float16
BF16 = mybir.dt.bfloat16
BF16_NP = ml_dtypes.bfloat16


def build_nc(reps: int = 1, loop_n: int = 0, phase1: bool = True, phase2: bool = True):
    """Build the per-core Bass kernel.

    `reps`: unrolled repetitions of the body.  `loop_n` > 0 instead wraps the
    body in a hardware For loop with that many iterations (timing builds).
    """
    from contextlib import ExitStack

    nc = bacc.Bacc("TRN2", target_bir_lowering=False, debug=False, num_devices=NC)
    qt = nc.dram_tensor("qt", [D, S], BF16, kind="ExternalInput").ap()
    kt = nc.dram_tensor("kt", [D, S], BF16, kind="ExternalInput").ap()
    vt = nc.dram_tensor("vt", [D, S], BF16, kind="ExternalInput").ap()
    wqk = nc.dram_tensor("wqk", [P, DO, 2, P], BF16, kind="ExternalInput").ap()
    wv = nc.dram_tensor("wv", [P, DO, P], BF16, kind="ExternalInput").ap()
    wot = nc.dram_tensor("wot", [P, D], BF16, kind="ExternalInput").ap()
    bqk = nc.dram_tensor("bqk", [P, 2], F32, kind="ExternalInput").ap()
    out = nc.dram_tensor("out", [S, D], F32, kind="ExternalOutput").ap()
    # The neuron NEFF cache keys on the HLO signature only (the Bass IR rides
    # out-of-band), so distinct kernel builds with identical I/O signatures can
    # collide with stale cache entries.  A version/config-sized dummy output
    # makes every build's signature unique.
    _KVER = 17
    vw = 16 + 64 * _KVER + 4 * loop_n + reps + (0 if phase1 else 1) + (0 if phase2 else 2)
    ver = nc.dram_tensor("ver", [1, vw], F32, kind="ExternalOutput").ap()

    qt_r = qt.rearrange("(o p) s -> p o s", p=P)
    kt_r = kt.rearrange("(o p) s -> p o s", p=P)
    vt_r = vt.rearrange("(o p) s -> p o s", p=P)
    out_r = out.rearrange("(so p) m -> p so m", p=P)  # [128, 32, 1024]

    with tile.TileContext(nc) as tc, ExitStack() as ctx:
        const = ctx.enter_context(tc.tile_pool(name="const", bufs=1))
        pers = ctx.enter_context(tc.tile_pool(name="pers", bufs=1))
        pin = ctx.enter_context(tc.tile_pool(name="pin", bufs=2))
        pP = ctx.enter_context(tc.tile_pool(name="pP", bufs=5))
        pout = ctx.enter_context(tc.tile_pool(name="pout", bufs=3))
        prr = ctx.enter_context(tc.tile_pool(name="prr", bufs=2))
        # PSUM: sA 4 banks + sB 2 banks + oT 1 bank + pv 1 bank = 8 banks
        psA = ctx.enter_context(tc.tile_pool(name="psA", bufs=1, space="PSUM"))
        psB = ctx.enter_context(tc.tile_pool(name="psB", bufs=1, space="PSUM"))
        psO = ctx.enter_context(tc.tile_pool(name="psO", bufs=1, space="PSUM"))
        psV = ctx.enter_context(tc.tile_pool(name="psV", bufs=1, space="PSUM"))

        wqk_sb = const.tile([P, DO, 2, P], BF16)
        nc.sync.dma_start(wqk_sb[:], wqk)
        wv_sb = const.tile([P, DO, P], BF16)
        nc.sync.dma_start(wv_sb[:], wv)
        wot_sb = const.tile([P, D], BF16)
        nc.sync.dma_start(wot_sb[:], wot)
        bqk_sb = const.tile([P, 2], F32)
        nc.sync.dma_start(bqk_sb[:], bqk)
        ones_sb = const.tile([1, DK], F32)
        nc.vector.memset(ones_sb[:], 1.0)
        ver_sb = const.tile([1, vw], F32)
        nc.vector.memset(ver_sb[:], float(vw))
        nc.sync.dma_start(ver, ver_sb[:])

        def body():
            qT = pers.tile([P, S], BF16, tag="qT", name="qT")
            kT = pers.tile([P, S], BF16, tag="kT", name="kT")
            vx0 = pers.tile([P, NTB, DK + 1], F16, tag="vx0", name="vx0")
            vx1 = pers.tile([P, NTB, DK + 1], F16, tag="vx1", name="vx1")
            xT = pers.tile([P, S], BF16, tag="xT", name="xT")
            nc.vector.memset(vx0[:, :, DK], 1.0)
            nc.vector.memset(vx1[:, :, DK], 1.0)

            # ---------------- phase 1: projections ----------------
            for sb in range(NSB if phase1 else 0):
                s0 = sb * SB
                qs = pin.tile([P, DO, SB], BF16, tag="qs", name="qs")
                nc.sync.dma_start(qs[:], qt_r[:, :, s0 : s0 + SB])
                ks = pin.tile([P, DO, SB], BF16, tag="ks", name="ks")
                nc.sync.dma_start(ks[:], kt_r[:, :, s0 : s0 + SB])
                vs = pin.tile([P, DO, SB], BF16, tag="vs", name="vs")
                nc.sync.dma_start(vs[:], vt_r[:, :, s0 : s0 + SB])

                # q/k projections: M=128 (both heads' j in one stationary tile)
                pq = psA.tile([P, 4 * SB], F32, tag="sA", name="pq")[:, 0:SB]
                pk = psB.tile([P, 2 * SB], F32, tag="sB", name="pk")[:, 0:SB]
                for o in range(DO):
                    nc.tensor.matmul(
                        pq, wqk_sb[:, o, 0, :], qs[:, o],
                        start=(o == 0), stop=(o == DO - 1),
                    )
                for o in range(DO):
                    nc.tensor.matmul(
                        pk, wqk_sb[:, o, 1, :], ks[:, o],
                        start=(o == 0), stop=(o == DO - 1),
                    )
                nc.vector.tensor_scalar(
                    qT[:, s0 : s0 + SB], pq,
                    bqk_sb[:, 0:1], 0.125,
                    mybir.AluOpType.add, mybir.AluOpType.mult,
                )
                nc.vector.tensor_scalar(
                    kT[:, s0 : s0 + SB], pk,
                    bqk_sb[:, 1:2], None,
                    mybir.AluOpType.add,
                )

                # v directly in [t, j] layout, j = h0 dk | h1 dk
                for tb in range(SB // TB):
                    # alternate between the two 1-bank slots (oT is idle in phase 1)
                    if tb % 2 == 0:
                        pv = psV.tile([P, SB], F32, tag="pv", name="pvv")
                    else:
                        pv = psO.tile([P, SB], F32, tag="oT", name="pvo")
                    pvv = pv[:, 0:P]
                    for o in range(DO):
                        nc.tensor.matmul(
                            pvv, vs[:, o, tb * TB : (tb + 1) * TB], wv_sb[:, o],
                            start=(o == 0), stop=(o == DO - 1),
                        )
                    tbg = sb * (SB // TB) + tb
                    nc.vector.tensor_copy(vx0[:, tbg, 0:DK], pvv[:, 0:DK])
                    nc.vector.tensor_copy(vx1[:, tbg, 0:DK], pvv[:, DK:P])

            # -------- phase 2: attention + output projection, fused per s-block -----
            for sb in range(NSB if phase2 else 0):
                s0 = sb * SB
                for h in range(HL):
                    vx = vx0 if h == 0 else vx1
                    oT = psO.tile([DK + 1, SB], F32, tag="oT", name="oT")
                    # software-pipelined: PV for group g issues a few groups late
                    # so the PE always has independent QK work while ACT does exp
                    starts = []
                    t = 0
                    for g in GROUPS:
                        starts.append(t)
                        t += g
                    pts = [None] * len(GROUPS)
                    NG = len(GROUPS)
                    PV_LAG = 3

                    def do_pv(gi):
                        g, gs = GROUPS[gi], starts[gi]
                        for i in range(g):
                            tb = gs + i
                            nc.tensor.matmul(
                                oT, vx[:, tb, :], pts[gi][:, i * SB : (i + 1) * SB],
                                start=(tb == 0), stop=(tb == NTB - 1),
                            )

                    for gi, g in enumerate(GROUPS):
                        gs = starts[gi]
                        pool, width = (psB, 2 * SB) if g == 2 else (psA, 4 * SB)
                        sc = pool.tile([P, width], F32, tag=("sB" if g == 2 else "sA"), name="sc")
                        for i in range(g):
                            tb = gs + i
                            nc.tensor.matmul(
                                sc[:, i * SB : (i + 1) * SB],
                                kT[h * DK : (h + 1) * DK, tb * TB : (tb + 1) * TB],
                                qT[h * DK : (h + 1) * DK, s0 : s0 + SB],
                                start=True, stop=True,
                            )
                        pt = pP.tile([P, 4 * SB], F16, tag="P", name="pt")
                        nc.scalar.activation(
                            pt[:, : g * SB], sc[:, : g * SB],
                            mybir.ActivationFunctionType.Exp,
                        )
                        pts[gi] = pt
                        if gi >= PV_LAG:
                            do_pv(gi - PV_LAG)
                    for gi in range(NG - PV_LAG, NG):
                        do_pv(gi)

                    # r = 1/l ; broadcast over 64 partitions ; xT = oT * r
                    r_t = prr.tile([1, SB], F32, tag="r", name="r_t")
                    r_s = prr.tile([1, SB], F32, tag="rs", name="r_s")
                    l_t = prr.tile([1, SB], F32, tag="lt", name="l_t")
                    # custom-DVE reciprocal mis-reads PSUM operands: stage l in SBUF
                    nc.vector.tensor_copy(l_t[:], oT[DK : DK + 1, :])
                    nc.vector.reciprocal_approx_accurate(r_t[:], l_t[:], r_s[:])
                    bc = psV.tile([DK, SB], F32, tag="pv", name="bc")
                    nc.tensor.matmul(bc, ones_sb[:], r_t[:], start=True, stop=True)
                    # DVE has a single PSUM read port: stage the broadcast in SBUF
                    bc_sb = prr.tile([DK, SB], F32, tag="bcs", name="bc_sb")
                    nc.vector.tensor_copy(bc_sb[:], bc)
                    nc.vector.tensor_tensor(
                        xT[h * DK : (h + 1) * DK, s0 : s0 + SB],
                        oT[0:DK, :], bc_sb[:], mybir.AluOpType.mult,
                    )

                # output projection for this s-block: single K=128 matmul per tile
                for si in range(SB // P):
                    so = sb * (SB // P) + si
                    for mb in range(2):
                        m0 = mb * SB
                        op = psV.tile([P, SB], F32, tag="pv", name="op")
                        nc.tensor.matmul(
                            op, xT[:, so * P : (so + 1) * P],
                            wot_sb[:, m0 : m0 + SB],
                            start=True, stop=True,
                        )
                        ob = pout.tile([P, SB], F32, tag="ob", name="ob")
                        nc.vector.tensor_copy(ob[:], op)
                        nc.sync.dma_start(out_r[:, so, m0 : m0 + SB], ob[:])

        if loop_n > 0:
            with tc.For_i(0, loop_n, 1):
                body()
        else:
            for _ in range(reps):
                body()

    nc.finalize()
    return nc


def _pack_core_inputs(c, QT, KT, VT, Wq, bq, Wk, bk, Wv, Wo):
    """Per-core input dict (core c owns heads 2c, 2c+1)."""
    h0 = HL * c
    # [p, o, j] with j = h*64 + dk (both heads side by side in the M dim)
    wq = Wq[h0 : h0 + HL].reshape(HL, DO, P, DK).transpose(2, 1, 0, 3).reshape(P, DO, P)
    wk = Wk[h0 : h0 + HL].reshape(HL, DO, P, DK).transpose(2, 1, 0, 3).reshape(P, DO, P)
    wqk = np.stack([wq, wk], axis=2).astype(BF16_NP)  # [p, o, qk, j]
    wv = (
        Wv[h0 : h0 + HL].reshape(HL, DO, P, DK).transpose(2, 1, 0, 3).reshape(P, DO, P)
    ).astype(BF16_NP)
    wot = np.ascontiguousarray(
        Wo[:, h0 * DK : (h0 + HL) * DK].T
    ).astype(BF16_NP)  # [j, m]
    bqk = np.stack(
        [np.concatenate([bq[h0], bq[h0 + 1]]), np.concatenate([bk[h0], bk[h0 + 1]])],
        axis=1,
    ).astype(np.float32)  # [128, 2]
    return {
        "qt": QT, "kt": KT, "vt": VT,
        "wqk": np.ascontiguousarray(wqk),
        "wv": np.ascontiguousarray(wv),
        "wot": np.ascontiguousarray(wot),
        "bqk": np.ascontiguousarray(bqk),
    }


def make_in_maps(Q, K, V, Wq, bq, Wk, bk, Wv, bv, Wo, bo):
    QT = np.ascontiguousarray(Q.T).astype(BF16_NP)
    KT = np.ascontiguousarray(K.T).astype(BF16_NP)
    VT = np.ascontiguousarray(V.T).astype(BF16_NP)
    return [
        _pack_core_inputs(c, QT, KT, VT, Wq, bq, Wk, bk, Wv, Wo) for c in range(NC)
    ]


def host_combine(partials, Wq, bv, Wo, bo):
    total = np.zeros((S, D), np.float32)
    for p in partials:
        total += p
    # v-bias passes through softmax exactly as +bv on the concat features
    total += bv.reshape(-1).astype(np.float32) @ Wo.T.astype(np.float32) + bo
    return total


_NC_CACHE = {}


def _get_nc(reps=1):
    if reps not in _NC_CACHE:
        _NC_CACHE[reps] = build_nc(reps)
    return _NC_CACHE[reps]


def kernel(Q, K, V, Wq, bq, Wk, bk, Wv, bv, Wo, bo):
    args = [np.asarray(x) for x in (Q, K, V, Wq, bq, Wk, bk, Wv, bv, Wo, bo)]
    Q, K, V, Wq, bq, Wk, bk, Wv, bv, Wo, bo = args
    nc = _get_nc()
    in_maps = make_in_maps(Q, K, V, Wq, bq, Wk, bk, Wv, bv, Wo, bo)
    res = run_bass_kernel_spmd(nc, in_maps, core_ids=list(range(NC)))
    partials = [res.results[c]["out"] for c in range(NC)]
    return host_combine(partials, Wq, bv, Wo, bo)



# revision 20
# speedup vs baseline: 1.3769x; 1.2478x over previous
"""Trainium2 Bass kernel: 16-head MHA (S=4096, D=1024) sharded 2 heads/core over 8 cores.

Per-core plan (heads h0=2c, h1=2c+1), v3 "round" architecture:
  - layouts: qT/xT [j, S] bf16 with j = h*64+dk on all 128 partitions.  kT
    stored per head as ZERO-PADDED K=128 tiles (kT0 rows 64:128 = 0, kT1 rows
    0:64 = 0): QK becomes a full-K matmul against the combined [h0|h1] qT (the
    zeros annihilate the other head) — HW-measured 163ns/MM vs 296ns at K=64.
    v in [t, (h, dk)] layout as vx [128, 32, 2, 65] fp16 (ones column 64 =
    softmax denominator).  v bias applied on host (exact: softmax rows sum to 1).
  - prologue: k and v projected for the full sequence (packed M=128 matmuls:
    both heads in one stationary tile), plus q for s-block 0.
  - steady state (per s-block, both heads together in 32 rounds of one t-block):
    round tb: QK h0 -> slot[:, 0, :], QK h1 -> slot[:, 1, :]; ONE ScalarE exp
    covers both heads [128, 1024] psum -> fp16 pt tile; PV (vx.T @ P per head,
    K=128) lags LAG rounds behind.  ScalarE is the bottleneck engine (~1040ns
    exp per round vs ~600ns PE), so the remaining PE work rides in the slack as
    interleaved work items: r-broadcast matmuls + output projection of the
    PREVIOUS s-block (single K=128 matmul per 128x512 tile) and the q
    projection of the NEXT s-block.
  - tails (end of each s-block): l=oT[64]; r=1/l via fast-reciprocal DVE op
    (SBUF-staged); r broadcast across 64 partitions with a K=1 matmul into a
    shared psum bank (h0 on partitions 0:64, h1 on 64:128); xT = oT * r (bf16)
    -> xT[j, S].  PSUM: 2+2 score slots, oT_h0, oT_h1, proj bank, bc/outproj
    bank = 8.
  - host: sum the 8 partial outputs + bias terms.
"""

import sys

for _p in ("/opt/trn_rl_repo",):
    if _p not in sys.path:
        sys.path.insert(0, _p)

import numpy as np
import ml_dtypes

import concourse.bass as bass  # noqa: F401
import concourse.mybir as mybir
import concourse.tile as tile
from concourse import bacc
from concourse.bass_utils import run_bass_kernel_spmd

P = 128
S = 4096
D = 1024
H = 16
DK = 64
HL = 2            # heads per core
NC = 8            # cores
SB = 512          # s-block width
NSB = S // SB     # 8
TB = 128          # t-block (scores partition dim)
NTB = S // TB     # 32
DO = D // P       # 8 d-chunks
LAG = 6           # PV rounds behind QK/exp (must exceed the 5 oT-reading
                  # work items of the previous s-block, emitted in rounds 0-4)

F32 = mybir.dt.float32
F16 = mybir.dt.float16
BF16 = mybir.dt.bfloat16
BF16_NP = ml_dtypes.bfloat16


def build_nc(reps: int = 1, loop_n: int = 0, phase1: bool = True, phase2: bool = True):
    """Build the per-core Bass kernel.

    `reps`: unrolled repetitions of the body.  `loop_n` > 0 instead wraps the
    body in a hardware For loop with that many iterations (timing builds).
    `phase1`/`phase2` gate the prologue / main loop for timing experiments.
    """
    from contextlib import ExitStack

    nc = bacc.Bacc("TRN2", target_bir_lowering=False, debug=False, num_devices=NC)
    qt = nc.dram_tensor("qt", [D, S], BF16, kind="ExternalInput").ap()
    kt = nc.dram_tensor("kt", [D, S], BF16, kind="ExternalInput").ap()
    vt = nc.dram_tensor("vt", [D, S], BF16, kind="ExternalInput").ap()
    wqk = nc.dram_tensor("wqk", [P, DO, 2, P], BF16, kind="ExternalInput").ap()
    wv = nc.dram_tensor("wv", [P, DO, P], BF16, kind="ExternalInput").ap()
    wot = nc.dram_tensor("wot", [P, D], BF16, kind="ExternalInput").ap()
    bqk = nc.dram_tensor("bqk", [P, 2], F32, kind="ExternalInput").ap()
    out = nc.dram_tensor("out", [S, D], F32, kind="ExternalOutput").ap()
    # The neuron NEFF cache keys on the HLO signature only (the Bass IR rides
    # out-of-band), so distinct kernel builds with identical I/O signatures can
    # collide with stale cache entries.  A version/config-sized dummy output
    # makes every build's signature unique.
    _KVER = 19
    vw = 16 + 64 * _KVER + 4 * loop_n + reps + (0 if phase1 else 1) + (0 if phase2 else 2)
    ver = nc.dram_tensor("ver", [1, vw], F32, kind="ExternalOutput").ap()

    qt_r = qt.rearrange("(o p) s -> p o s", p=P)
    kt_r = kt.rearrange("(o p) s -> p o s", p=P)
    vt_r = vt.rearrange("(o p) s -> p o s", p=P)
    out_r = out.rearrange("(so p) m -> p so m", p=P)  # [128, 32, 1024]

    with tile.TileContext(nc) as tc, ExitStack() as ctx:
        const = ctx.enter_context(tc.tile_pool(name="const", bufs=1))
        pers = ctx.enter_context(tc.tile_pool(name="pers", bufs=1))
        pin = ctx.enter_context(tc.tile_pool(name="pin", bufs=2))
        pP = ctx.enter_context(tc.tile_pool(name="pP", bufs=LAG + 3))
        pout = ctx.enter_context(tc.tile_pool(name="pout", bufs=3))
        prr = ctx.enter_context(tc.tile_pool(name="prr", bufs=2))
        # PSUM (8 banks): slotA 2 + slotB 2 + oT0 1 + oT1 1 + proj 1 + bc/op 1
        psA = ctx.enter_context(tc.tile_pool(name="psA", bufs=1, space="PSUM"))
        psB = ctx.enter_context(tc.tile_pool(name="psB", bufs=1, space="PSUM"))
        psO0 = ctx.enter_context(tc.tile_pool(name="psO0", bufs=1, space="PSUM"))
        psO1 = ctx.enter_context(tc.tile_pool(name="psO1", bufs=1, space="PSUM"))
        psPj = ctx.enter_context(tc.tile_pool(name="psPj", bufs=1, space="PSUM"))
        psX = ctx.enter_context(tc.tile_pool(name="psX", bufs=1, space="PSUM"))

        wqk_sb = const.tile([P, DO, 2, P], BF16)
        nc.sync.dma_start(wqk_sb[:], wqk)
        wv_sb = const.tile([P, DO, P], BF16)
        nc.sync.dma_start(wv_sb[:], wv)
        wot_sb = const.tile([P, D], BF16)
        nc.sync.dma_start(wot_sb[:], wot)
        bqk_sb = const.tile([P, 2], F32)
        nc.sync.dma_start(bqk_sb[:], bqk)
        ones_sb = const.tile([1, DK], F32)
        nc.vector.memset(ones_sb[:], 1.0)
        ver_sb = const.tile([1, vw], F32)
        nc.vector.memset(ver_sb[:], float(vw))
        nc.sync.dma_start(ver, ver_sb[:])

        def body():
            qT = pers.tile([P, S], BF16, tag="qT", name="qT")
            # per-head zero-padded K=128 stationary tiles: zeros in the other
            # head's rows make QK a full-K matmul (HW: 163ns/MM vs 296 at K=64)
            # while the moving operand stays the combined [h0|h1] qT.
            kT0 = pers.tile([P, S], BF16, tag="kT0", name="kT0")
            kT1 = pers.tile([P, S], BF16, tag="kT1", name="kT1")
            vx = pers.tile([P, NTB, HL, DK + 1], F16, tag="vx", name="vx")
            xT = pers.tile([P, S], BF16, tag="xT", name="xT")
            nc.gpsimd.memset(kT0[DK:P, :], 0.0)
            nc.gpsimd.memset(kT1[0:DK, :], 0.0)
            nc.vector.memset(vx[:, :, :, DK], 1.0)

            def proj_q(sb):
                """DMA + project q for s-block sb -> qT[:, sb*SB:...] (as emit list)."""
                s0 = sb * SB
                items = []
                qs = pin.tile([P, DO, SB], BF16, tag="qs", name="qs")
                items.append(lambda: nc.sync.dma_start(qs[:], qt_r[:, :, s0 : s0 + SB]))
                pq = psPj.tile([P, SB], F32, tag="pj", name="pq")
                for o in range(DO):
                    items.append(
                        lambda o=o: nc.tensor.matmul(
                            pq, wqk_sb[:, o, 0, :], qs[:, o],
                            start=(o == 0), stop=(o == DO - 1),
                        )
                    )
                items.append(
                    lambda: nc.vector.tensor_scalar(
                        qT[:, s0 : s0 + SB], pq, bqk_sb[:, 0:1], 0.125,
                        mybir.AluOpType.add, mybir.AluOpType.mult,
                    )
                )
                return items

            # ---------------- prologue: k/v for full sequence, q for sb 0 ----
            if phase1:
                for sb in range(NSB):
                    s0 = sb * SB
                    ks = pin.tile([P, DO, SB], BF16, tag="ks", name="ks")
                    nc.sync.dma_start(ks[:], kt_r[:, :, s0 : s0 + SB])
                    vs = pin.tile([P, DO, SB], BF16, tag="vs", name="vs")
                    nc.sync.dma_start(vs[:], vt_r[:, :, s0 : s0 + SB])

                    pk = psPj.tile([P, SB], F32, tag="pj", name="pk")
                    for o in range(DO):
                        nc.tensor.matmul(
                            pk, wqk_sb[:, o, 1, :], ks[:, o],
                            start=(o == 0), stop=(o == DO - 1),
                        )
                    nc.vector.tensor_scalar(
                        kT0[0:DK, s0 : s0 + SB], pk[0:DK, :], bqk_sb[0:DK, 1:2], None,
                        mybir.AluOpType.add,
                    )
                    nc.vector.tensor_scalar(
                        kT1[DK:P, s0 : s0 + SB], pk[DK:P, :], bqk_sb[DK:P, 1:2], None,
                        mybir.AluOpType.add,
                    )

                    # v in [t, (h, dk)] layout; slots A/B are idle in the prologue
                    for tb in range(SB // TB):
                        pool = psA if tb % 2 == 0 else psB
                        tg = "sA" if tb % 2 == 0 else "sB"
                        pv = pool.tile([P, HL, SB], F32, tag=tg, name="pv")
                        pvv = pv[:, 0, 0:P]
                        for o in range(DO):
                            nc.tensor.matmul(
                                pvv, vs[:, o, tb * TB : (tb + 1) * TB], wv_sb[:, o],
                                start=(o == 0), stop=(o == DO - 1),
                            )
                        tbg = sb * (SB // TB) + tb
                        nc.vector.tensor_copy(vx[:, tbg, :, 0:DK], pvv[:])
                for it in proj_q(0):
                    it()

            # ---------------- steady state: 8 s-blocks x 32 rounds ----------
            for sb in range(NSB if phase2 else 0):
                s0 = sb * SB

                # deferred PE/DVE work from the previous s-block + next q proj,
                # one item per round starting at round 0
                items = []
                if sb > 0:
                    pb = sb - 1
                    p0 = pb * SB
                    bc = psX.tile([P, SB], F32, tag="bc", name="bc")
                    for h, (rt, oT) in enumerate(zip(r_ts, oTs)):
                        items.append(
                            lambda h=h, rt=rt: nc.tensor.matmul(
                                bc[h * DK : (h + 1) * DK, :], ones_sb[:], rt[:],
                                start=True, stop=True,
                            )
                        )
                    bc_sb = prr.tile([P, SB], F32, tag="bcs", name="bc_sb")
                    items.append(lambda: nc.vector.tensor_copy(bc_sb[:], bc))
                    for h, oT in enumerate(oTs):
                        items.append(
                            lambda h=h, oT=oT: nc.vector.tensor_tensor(
                                xT[h * DK : (h + 1) * DK, p0 : p0 + SB],
                                oT[0:DK, :], bc_sb[h * DK : (h + 1) * DK, :],
                                mybir.AluOpType.mult,
                            )
                        )

                    def outproj(si, mb, pb=pb, p0=p0):
                        so = pb * (SB // P) + si
                        m0 = mb * SB
                        op = psX.tile([P, SB], F32, tag="bc", name="op")
                        nc.tensor.matmul(
                            op, xT[:, so * P : (so + 1) * P], wot_sb[:, m0 : m0 + SB],
                            start=True, stop=True,
                        )
                        ob = pout.tile([P, SB], F32, tag="ob", name="ob")
                        nc.vector.tensor_copy(ob[:], op)
                        nc.sync.dma_start(out_r[:, so, m0 : m0 + SB], ob[:])

                    for si in range(SB // P):
                        for mb in range(2):
                            items.append(lambda si=si, mb=mb: outproj(si, mb))
                if sb < NSB - 1:
                    items.extend(proj_q(sb + 1))

                oT0 = psO0.tile([DK + 1, SB], F32, tag="oT0", name="oT0")
                oT1 = psO1.tile([DK + 1, SB], F32, tag="oT1", name="oT1")
                oTs = (oT0, oT1)
                pts = [None] * NTB

                def do_pv(tbl):
                    for h, oT in enumerate(oTs):
                        nc.tensor.matmul(
                            oT, vx[:, tbl, h, :], pts[tbl][:, h, :],
                            start=(tbl == 0), stop=(tbl == NTB - 1),
                        )

                for tb in range(NTB):
                    pool = psA if tb % 2 == 0 else psB
                    tg = "sA" if tb % 2 == 0 else "sB"
                    sc = pool.tile([P, HL, SB], F32, tag=tg, name="sc")
                    for h, kTh in enumerate((kT0, kT1)):
                        nc.tensor.matmul(
                            sc[:, h, :],
                            kTh[:, tb * TB : (tb + 1) * TB],
                            qT[:, s0 : s0 + SB],
                            start=True, stop=True,
                        )
                    pt = pP.tile([P, HL, SB], F16, tag="P", name="pt")
                    nc.scalar.activation(
                        pt[:], sc[:], mybir.ActivationFunctionType.Exp,
                    )
                    pts[tb] = pt
                    # one deferred work item per round; the items that read the
                    # previous oT banks MUST be emitted before do_pv(0) below
                    # resets them (rounds 0..4 < LAG)
                    if items:
                        items.pop(0)()
                    if tb >= LAG:
                        do_pv(tb - LAG)
                for tbl in range(NTB - LAG, NTB):
                    do_pv(tbl)
                for it in items:
                    it()

                # tails: r = 1/l per head (consumed by bc in the next s-block)
                r_ts = []
                for h, oT in enumerate(oTs):
                    l_t = prr.tile([1, SB], F32, tag=f"lt{h}", name="l_t")
                    r_t = prr.tile([1, SB], F32, tag=f"rt{h}", name="r_t")
                    r_s = prr.tile([1, SB], F32, tag=f"rs{h}", name="r_s")
                    # custom-DVE reciprocal mis-reads PSUM operands: stage in SBUF
                    nc.vector.tensor_copy(l_t[:], oT[DK : DK + 1, :])
                    nc.vector.reciprocal_approx_accurate(r_t[:], l_t[:], r_s[:])
                    r_ts.append(r_t)

            # final s-block's bc + xT + output projection (nothing to hide under)
            if phase2:
                pb = NSB - 1
                p0 = pb * SB
                bc = psX.tile([P, SB], F32, tag="bc", name="bc")
                for h, rt in enumerate(r_ts):
                    nc.tensor.matmul(
                        bc[h * DK : (h + 1) * DK, :], ones_sb[:], rt[:],
                        start=True, stop=True,
                    )
                bc_sb = prr.tile([P, SB], F32, tag="bcs", name="bc_sb")
                nc.vector.tensor_copy(bc_sb[:], bc)
                for h, oT in enumerate(oTs):
                    nc.vector.tensor_tensor(
                        xT[h * DK : (h + 1) * DK, p0 : p0 + SB],
                        oT[0:DK, :], bc_sb[h * DK : (h + 1) * DK, :],
                        mybir.AluOpType.mult,
                    )
                for si in range(SB // P):
                    so = pb * (SB // P) + si
                    for mb in range(2):
                        m0 = mb * SB
                        op = psX.tile([P, SB], F32, tag="bc", name="op")
                        nc.tensor.matmul(
                            op, xT[:, so * P : (so + 1) * P], wot_sb[:, m0 : m0 + SB],
                            start=True, stop=True,
                        )
                        ob = pout.tile([P, SB], F32, tag="ob", name="ob")
                        nc.vector.tensor_copy(ob[:], op)
                        nc.sync.dma_start(out_r[:, so, m0 : m0 + SB], ob[:])

        if loop_n > 0:
            with tc.For_i(0, loop_n, 1):
                body()
        else:
            for _ in range(reps):
                body()

    nc.finalize()
    return nc


def _pack_core_inputs(c, QT, KT, VT, Wq, bq, Wk, bk, Wv, Wo):
    """Per-core input dict (core c owns heads 2c, 2c+1)."""
    h0 = HL * c
    # [p, o, j] with j = h*64 + dk (both heads side by side in the M dim)
    wq = Wq[h0 : h0 + HL].reshape(HL, DO, P, DK).transpose(2, 1, 0, 3).reshape(P, DO, P)
    wk = Wk[h0 : h0 + HL].reshape(HL, DO, P, DK).transpose(2, 1, 0, 3).reshape(P, DO, P)
    wqk = np.stack([wq, wk], axis=2).astype(BF16_NP)  # [p, o, qk, j]
    wv = (
        Wv[h0 : h0 + HL].reshape(HL, DO, P, DK).transpose(2, 1, 0, 3).reshape(P, DO, P)
    ).astype(BF16_NP)
    wot = np.ascontiguousarray(
        Wo[:, h0 * DK : (h0 + HL) * DK].T
    ).astype(BF16_NP)  # [j, m]
    bqk = np.stack(
        [np.concatenate([bq[h0], bq[h0 + 1]]), np.concatenate([bk[h0], bk[h0 + 1]])],
        axis=1,
    ).astype(np.float32)  # [128, 2]
    return {
        "qt": QT, "kt": KT, "vt": VT,
        "wqk": np.ascontiguousarray(wqk),
        "wv": np.ascontiguousarray(wv),
        "wot": np.ascontiguousarray(wot),
        "bqk": np.ascontiguousarray(bqk),
    }


def make_in_maps(Q, K, V, Wq, bq, Wk, bk, Wv, bv, Wo, bo):
    QT = np.ascontiguousarray(Q.T).astype(BF16_NP)
    KT = np.ascontiguousarray(K.T).astype(BF16_NP)
    VT = np.ascontiguousarray(V.T).astype(BF16_NP)
    return [
        _pack_core_inputs(c, QT, KT, VT, Wq, bq, Wk, bk, Wv, Wo) for c in range(NC)
    ]


def host_combine(partials, Wq, bv, Wo, bo):
    total = np.zeros((S, D), np.float32)
    for p in partials:
        total += p
    # v-bias passes through softmax exactly as +bv on the concat features
    total += bv.reshape(-1).astype(np.float32) @ Wo.T.astype(np.float32) + bo
    return total


_NC_CACHE = {}


def _get_nc(reps=1):
    if reps not in _NC_CACHE:
        _NC_CACHE[reps] = build_nc(reps)
    return _NC_CACHE[reps]


def kernel(Q, K, V, Wq, bq, Wk, bk, Wv, bv, Wo, bo):
    args = [np.asarray(x) for x in (Q, K, V, Wq, bq, Wk, bk, Wv, bv, Wo, bo)]
    Q, K, V, Wq, bq, Wk, bk, Wv, bv, Wo, bo = args
    nc = _get_nc()
    in_maps = make_in_maps(Q, K, V, Wq, bq, Wk, bk, Wv, bv, Wo, bo)
    res = run_bass_kernel_spmd(nc, in_maps, core_ids=list(range(NC)))
    partials = [res.results[c]["out"] for c in range(NC)]
    return host_combine(partials, Wq, bv, Wo, bo)
